# revision 13
# baseline (speedup 1.0000x reference)
"""Bahdanau additive attention on 8 TRN2 NeuronCores, pure data parallel.

Per core (B_loc = 256 batch rows, two 128-row chunks):
  h1 = features @ W1                    -- fp16 (e<256) + fp8-feat x fp16-W1
                                           (e>=256) matmuls, fp32 PSUM accum
  z  = h1 + hidden@W2 + b1 + b2         -- h2/biases folded into PSUM via PE
  t  = tanh(z)                          -- ACT, per s-pair [128,1024]
  scores = t @ Wv                       -- DVE TT-mult (2x) + TS accum (4x)
  p  = exp(scores - max)                -- unnormalized softmax weights
  ctx = (sum_s p_s * feat_s) / Z        -- DVE TS(4x) products + TT(2x) tree;
                                           last chunk partly via PE diag-MMs
                                           to hide the pipeline tail.

DMA is the binding resource: features are cast-loaded fp32->fp16 once
(HBM floor ~94us/core), and the on-chip xbar transposes (256B-packet
limited) are trimmed by transposing the top e-half as fp8 pairs packed
in fp16 atoms.  The fp8 e-range pairs (e=256+2j, 256+2j+1) land on
partition j; the matching W1 rows are pair-interleaved on the host
(W1bot input, fp16) so contraction stays consistent.  rel-err ~1.3e-2.
"""

import numpy as np

import concourse.bass as bass
import concourse.bacc as bacc
import concourse.mybir as mybir
import concourse.tile as tile
from concourse.bass_utils import run_bass_kernel_spmd

F8 = mybir.dt.float8e4
F16 = mybir.dt.float16
F32 = mybir.dt.float32
AX = mybir.AxisListType
ALU = mybir.AluOpType
ACTF = mybir.ActivationFunctionType

B, S, E, H, U = 2048, 64, 512, 512, 512
N_CORES = 8
BL = B // N_CORES          # 256 rows per core
NCHUNK = BL // 128         # 2 chunks of 128 rows
S_GRP = 8                  # s rows per cast/transpose group
N_GRP = S // S_GRP
EC = E // 128              # 4 contraction chunks
E8 = E // 2                # fp8 e-range start (e >= E8 is fp8)
HC = H // 128

_LAST_RESULTS = {}


def build_kernel(reps: int = 1) -> bacc.Bacc:
    import os
    feat_slots = int(os.environ.get("FEAT_SLOTS", "12"))
    featt_bufs = int(os.environ.get("FEATT_BUFS", "4"))
    t16_bufs = int(os.environ.get("T16_BUFS", "12"))
    zt_bufs = int(os.environ.get("ZT_BUFS", "3"))
    n_ctx_pe_tail = int(os.environ.get("N_CTX_PE", "48"))  # last-chunk PE ctx
    ctx_blk = int(os.environ.get("CTX_BLK", "8"))          # DVE ctx tree block

    nc = bacc.Bacc(target_bir_lowering=False)

    feat_d = nc.declare_dram_parameter("features", [BL, S, E], F32, isOutput=False)
    hid_d = nc.declare_dram_parameter("hidden", [BL, H], F32, isOutput=False)
    w1_d = nc.declare_dram_parameter("W1", [E, U], F32, isOutput=False)
    w1b_d = nc.declare_dram_parameter("W1bot", [128, 2, U], F16, isOutput=False)
    b1_d = nc.declare_dram_parameter("b1", [U], F32, isOutput=False)
    w2_d = nc.declare_dram_parameter("W2", [H, U], F32, isOutput=False)
    b2_d = nc.declare_dram_parameter("b2", [U], F32, isOutput=False)
    wv_d = nc.declare_dram_parameter("Wv", [U, 1], F32, isOutput=False)
    id_d = nc.declare_dram_parameter("ident", [128, 128], F16, isOutput=False)
    out_d = nc.declare_dram_parameter("out", [BL, E], F32, isOutput=True)

    with tile.TileContext(nc) as tc:
        with (
            tc.tile_pool(name="const", bufs=1) as cpool,
            tc.tile_pool(name="featn", bufs=feat_slots) as fpool,
            tc.tile_pool(name="feat8", bufs=3) as f8pool,
            tc.tile_pool(name="featT", bufs=featt_bufs) as tpool,
            tc.tile_pool(name="work", bufs=2) as wpool,
            tc.tile_pool(name="tanh", bufs=t16_bufs) as hpool,
            tc.tile_pool(name="ctxp", bufs=2) as xpool,
            tc.tile_pool(name="pz", bufs=zt_bufs, space="PSUM") as pz,
            tc.tile_pool(name="ph2", bufs=1, space="PSUM") as ph2,
            tc.tile_pool(name="pctx", bufs=1, space="PSUM") as pctx,
        ):
            # ---- constants / weights ----
            # top e-half of W1 (fp16, via cast-DMA); bottom half arrives
            # pre-interleaved from the host as W1bot.
            w1_sb = cpool.tile([128, 2, U], F16)
            nc.gpsimd.dma_start(
                w1_sb[:], w1_d[0:E8, :].rearrange("(c p) u -> p c u", p=128))
            w1b_sb = cpool.tile([128, 2, U], F16)
            nc.sync.dma_start(w1b_sb[:], w1b_d[:])
            w2_sb = cpool.tile([128, HC, U], F16)
            nc.gpsimd.dma_start(w2_sb[:], w2_d.rearrange("(c p) u -> p c u", p=128))
            ident = cpool.tile([128, 128], F16)
            nc.sync.dma_start(ident[:], id_d[:])
            ones1 = cpool.tile([1, 128], F16)
            nc.vector.memset(ones1[:], 1.0)
            b1row = cpool.tile([1, U], F16)
            nc.gpsimd.dma_start(b1row[:], b1_d.rearrange("(one u) -> one u", one=1))
            b2row = cpool.tile([1, U], F16)
            nc.gpsimd.dma_start(b2row[:], b2_d.rearrange("(one u) -> one u", one=1))
            # wv twice in one row so one [128,1024] broadcast serves s-pairs
            wv2_row = cpool.tile([1, 2, U], F16)
            nc.gpsimd.dma_start(wv2_row[:, 0, :], wv_d.rearrange("u one -> one u"))
            nc.gpsimd.dma_start(wv2_row[:, 1, :], wv_d.rearrange("u one -> one u"))

            ps_wv = pz.tile([128, 1024], F32, tag="pz")
            nc.tensor.matmul(ps_wv[:, 0:512], ones1[:], wv2_row[:, 0, :],
                             start=True, stop=True)
            nc.tensor.matmul(ps_wv[:, 512:1024], ones1[:], wv2_row[:, 1, :],
                             start=True, stop=True)
            wv2_rep = cpool.tile([128, 1024], F16)
            nc.scalar.activation(wv2_rep[:], ps_wv[:], ACTF.Copy)

            chunk_ids = [c for _ in range(reps) for c in range(NCHUNK)]
            n_steps = len(chunk_ids)

            feat16 = {}   # (step, group) -> fp16 tile [128, S_GRP, E]

            def issue_load(i, g):
                if (i, g) in feat16 or i >= n_steps:
                    return
                b0 = chunk_ids[i] * 128
                fg = fpool.tile([128, S_GRP, E], F16,
                                name=f"feat16_{i}_{g}", tag="feat16")
                nc.gpsimd.dma_start(
                    fg[:], feat_d[b0:b0 + 128, g * S_GRP:(g + 1) * S_GRP, :])
                feat16[(i, g)] = fg

            for g in range(N_GRP):
                issue_load(0, g)
            issue_load(1, 0)
            issue_load(1, 1)

            feat8 = {}    # (step, group) -> fp8 tile [128, S_GRP, 256]

            def issue_cast(i, g):
                # fp16 -> fp8 cast of the top e-half, on ACT
                if (i, g) in feat8 or i >= n_steps:
                    return
                f8 = f8pool.tile([128, S_GRP, E - E8], F8,
                                 name=f"feat8_{i}_{g}", tag="feat8")
                nc.scalar.activation(
                    f8[:], feat16[(i, g)][:, :, E8:E], ACTF.Copy)
                feat8[(i, g)] = f8

            featT16 = {}  # (step, group) -> [128, 2*S_GRP, 128] fp16
            featT8 = {}   # (step, group) -> fp8-pair view [128, S_GRP, 128, 2]

            def issue_transposes(i, g):
                # xbar needs a 2D-contiguous source; stage the low e-half
                # into a contiguous tile first (DVE copy, 4x mode).
                st = f8pool.tile([128, S_GRP, E8], F16,
                                 name=f"stage16_{i}_{g}", tag="stage16", bufs=3)
                nc.vector.tensor_copy(st[:], feat16[(i, g)][:, :, 0:E8])
                ft = tpool.tile([128, 2 * S_GRP, 128], F16,
                                name=f"featT16_{i}_{g}", tag="featT16")
                nc.sync.dma_start(ft[:], st[:], transpose=True)
                featT16[(i, g)] = ft
                f8t = tpool.tile([128, S_GRP, 128], F16,
                                 name=f"featT8_{i}_{g}", tag="featT8",
                                 bufs=featt_bufs)
                nc.sync.dma_start(f8t[:], feat8[(i, g)][:].bitcast(F16),
                                  transpose=True)
                featT8[(i, g)] = f8t[:].bitcast(F8).rearrange(
                    "p s (b two) -> p s b two", two=2)

            for i, c in enumerate(chunk_ids):
                b0 = c * 128
                last = (i == n_steps - 1)
                # casts for the first two groups (their loads landed first)
                issue_cast(i, 0)
                issue_cast(i, 1)

                # ---- h2 = hidden @ W2 + b1 + b2 (fp32 psum) ----
                hid16 = wpool.tile([128, H], F16, name=f"hid16_{i}", tag="hid16")
                nc.gpsimd.dma_start(hid16[:], hid_d[b0:b0 + 128, :])
                hidT = wpool.tile([128, HC, 128], F16, name=f"hidT_{i}", tag="hidT")
                nc.sync.dma_start(hidT[:], hid16[:], transpose=True)
                ps_h2 = ph2.tile([128, U], F32, tag="ph2")
                for k in range(HC):
                    nc.tensor.matmul(
                        ps_h2[:], hidT[:, k, :], w2_sb[:, k, :],
                        start=(k == 0), stop=False,
                    )
                nc.tensor.matmul(ps_h2[:], ones1[:], b1row[:], start=False, stop=False)
                nc.tensor.matmul(ps_h2[:], ones1[:], b2row[:], start=False, stop=True)
                h2_16 = wpool.tile([128, U], F16, name=f"h2_16_{i}", tag="h2_16")
                nc.scalar.activation(h2_16[:], ps_h2[:], ACTF.Copy)

                issue_transposes(i, 0)
                issue_transposes(i, 1)

                scores = wpool.tile([128, S], F32, name=f"scores_{i}", tag="scores")

                # ---- PE stream + tanh + scores, per s-pair ----
                for sp in range(S // 2):
                    s0 = sp * 2
                    g = s0 // S_GRP
                    if s0 % S_GRP == 0 and g + 2 < N_GRP:
                        # group boundary: stage group g+2 (cast + transposes)
                        issue_cast(i, g + 2)
                        issue_transposes(i, g + 2)
                    ps = pz.tile([128, 1024], F32, tag="pz")
                    for half in range(2):
                        s = s0 + half
                        sl = s % S_GRP
                        col = slice(half * 512, half * 512 + 512)
                        for k in range(2):
                            nc.tensor.matmul(
                                ps[:, col],
                                featT16[(i, g)][:, sl * 2 + k, :],
                                w1_sb[:, k, :],
                                start=(k == 0), stop=False,
                            )
                        f8v = featT8[(i, g)]
                        for par in range(2):
                            nc.tensor.matmul(
                                ps[:, col],
                                f8v[:, sl, :, par],
                                w1b_sb[:, par, :],
                                start=False, stop=False,
                            )
                        nc.tensor.matmul(
                            ps[:, col], ident[:], h2_16[:],
                            start=False, stop=True,
                        )
                    t16 = hpool.tile([128, 1024], F16)
                    nc.scalar.activation(t16[:], ps[:], ACTF.Tanh)
                    # scores: in-place wv multiply (TT 2x), then per-s
                    # tensor_scalar accumulate
                    nc.vector.tensor_mul(t16[:], t16[:], wv2_rep[:])
                    for half in range(2):
                        s = s0 + half
                        dump = hpool.tile([128, 512], F16, tag="dump", bufs=2)
                        nc.vector.tensor_scalar(
                            out=dump[:],
                            in0=t16[:, half * 512: half * 512 + 512],
                            scalar1=1.0, scalar2=None,
                            op0=ALU.mult, op1=ALU.add,
                            accum_out=scores[:, s:s + 1],
                        )

                # ---- prefetch loads for the next chunk ----
                for g in range(2, N_GRP):
                    issue_load(i + 1, g)

                # ---- softmax over s (unnormalized; 1/Z applied at end) ----
                negmax = wpool.tile([128, 1], F32)
                nc.vector.tensor_reduce(
                    out=negmax[:], in_=scores[:], axis=AX.X, op=ALU.max, negate=True,
                )
                probs = wpool.tile([128, S], F32)
                zsum = wpool.tile([128, 1], F32)
                nc.scalar.activation(
                    probs[:], scores[:], ACTF.Exp,
                    bias=negmax[:], scale=1.0, accum_out=zsum[:],
                )
                rz = wpool.tile([128, 1], F32)
                nc.vector.reciprocal(rz[:], zsum[:])

                # ---- context: sum_s p_s * feat_s ----
                n_pe = n_ctx_pe_tail if last else 0
                n_dve = S - n_pe

                dve_accs = []
                for blk_start in range(0, n_dve, ctx_blk):
                    blk = list(range(blk_start, min(blk_start + ctx_blk, n_dve)))
                    acc = xpool.tile([128, E], F16, tag="ctxacc", bufs=10,
                                     name=f"ctxacc_{i}_{blk_start}")
                    for j, s in enumerate(blk):
                        g, sl = s // S_GRP, s % S_GRP
                        fslice = feat16[(i, g)][:, sl, :]
                        if j == 0:
                            nc.vector.tensor_scalar_mul(
                                acc[:], fslice, probs[:, s:s + 1])
                        else:
                            tmp = xpool.tile([128, E], F16, tag="ctxtmp", bufs=2,
                                             name=f"ctxtmp_{i}_{blk_start}_{j}")
                            nc.vector.tensor_scalar_mul(
                                tmp[:], fslice, probs[:, s:s + 1])
                            nc.vector.tensor_add(acc[:], acc[:], tmp[:])
                    dve_accs.append(acc)

                # PE part (last chunk only): diag(p_s) matmuls into PSUM
                if n_pe > 0:
                    ctx_ps = pctx.tile([128, E], F32, tag="pctx")
                    diags = []
                    for s in range(n_dve, S):
                        dg = xpool.tile([128, 128], F16, tag="diag", bufs=6,
                                        name=f"diag_{i}_{s}")
                        nc.vector.tensor_scalar_mul(
                            dg[:], ident[:], probs[:, s:s + 1])
                        diags.append(dg)
                    for j, s in enumerate(range(n_dve, S)):
                        g, sl = s // S_GRP, s % S_GRP
                        nc.tensor.matmul(
                            ctx_ps[:], diags[j][:], feat16[(i, g)][:, sl, :],
                            start=(j == 0), stop=(s == S - 1),
                        )
                    ctx_pe16 = xpool.tile([128, E], F16, tag="ctxpe",
                                          name=f"ctxpe_{i}")
                    nc.scalar.activation(ctx_pe16[:], ctx_ps[:], ACTF.Copy)
                    dve_accs.append(ctx_pe16)

                total = dve_accs[0]
                for acc in dve_accs[1:]:
                    nc.vector.tensor_add(total[:], total[:], acc[:])
                ctx16 = wpool.tile([128, E], F16, name=f"ctx16_{i}", tag="ctx16")
                nc.vector.tensor_scalar_mul(ctx16[:], total[:], rz[:])
                nc.gpsimd.dma_start(out_d[b0:b0 + 128, :], ctx16[:])

                # chunk-after-next's first two groups (slots recycle groups
                # this chunk has fully consumed)
                issue_load(i + 2, 0)
                issue_load(i + 2, 1)

    nc.compile()
    return nc


def _prep_w1bot(W1: np.ndarray) -> np.ndarray:
    # W1bot[j, i, u] = W1[E8 + 2j + i, u] as fp16 (pair-interleave to match
    # the fp16-atom transpose of fp8 pairs)
    bot = W1[E8:E, :]                       # [256, U]
    il = bot.reshape(128, 2, U)             # rows (2j, 2j+1) -> [j, i, u]
    return np.ascontiguousarray(il.astype(np.float16))


def kernel(**inputs) -> np.ndarray:
    features = np.ascontiguousarray(np.asarray(inputs["features"], dtype=np.float32))
    hidden = np.ascontiguousarray(np.asarray(inputs["hidden"], dtype=np.float32))
    W1 = np.ascontiguousarray(np.asarray(inputs["W1"], dtype=np.float32))
    b1 = np.ascontiguousarray(np.asarray(inputs["b1"], dtype=np.float32))
    W2 = np.ascontiguousarray(np.asarray(inputs["W2"], dtype=np.float32))
    b2 = np.ascontiguousarray(np.asarray(inputs["b2"], dtype=np.float32))
    Wv = np.ascontiguousarray(np.asarray(inputs["Wv"], dtype=np.float32))
    # bv shifts every score equally; softmax is invariant to it.

    nc = build_kernel()
    ident = np.eye(128, dtype=np.float16)
    w1bot = _prep_w1bot(W1)
    in_maps = []
    for i in range(N_CORES):
        in_maps.append({
            "features": features[i * BL:(i + 1) * BL],
            "hidden": hidden[i * BL:(i + 1) * BL],
            "W1": W1, "W1bot": w1bot, "b1": b1, "W2": W2, "b2": b2, "Wv": Wv,
            "ident": ident,
        })
    import os as _os
    _tmpdir = _os.environ.get("BASS_TMPDIR") or None
    res = run_bass_kernel_spmd(nc, in_maps, core_ids=list(range(N_CORES)),
                               tmpdir=_tmpdir)
    _LAST_RESULTS["res"] = res
    if res.exec_time_ns is not None:
        print(f"HW exec time: {res.exec_time_ns} ns")
    out = np.concatenate([res.results[i]["out"] for i in range(N_CORES)], axis=0)
    return out.astype(np.float32)


# revision 15
# speedup vs baseline: 1.0723x; 1.0723x over previous
"""Bahdanau additive attention on 8 TRN2 NeuronCores, pure data parallel.

Per core (B_loc = 256 batch rows, two 128-row chunks):
  h1 = features @ W1                    -- fp16 (e<256) + fp8-feat x fp16-W1
                                           (e>=256) matmuls, fp32 PSUM accum
  z  = h1 + hidden@W2 + b1 + b2         -- h2/biases folded into PSUM via PE
  t  = tanh(z)                          -- ACT, per s-pair [128,1024]
  scores = t @ Wv                       -- DVE TT-mult (2x) + TS accum (4x)
  p  = exp(scores - max)                -- unnormalized softmax weights
  ctx = (sum_s p_s * feat_s) / Z        -- DVE TS(4x) products + TT(2x) tree;
                                           last chunk partly via PE diag-MMs
                                           to hide the pipeline tail.

DMA is the binding resource: features are cast-loaded fp32->fp16 once
(HBM floor ~94us/core), and the on-chip xbar transposes (256B-packet
limited) are trimmed by transposing the top e-half as fp8 pairs packed
in fp16 atoms.  The fp8 e-range pairs (e=256+2j, 256+2j+1) land on
partition j; the matching W1 rows are pair-interleaved on the host
(W1bot input, fp16) so contraction stays consistent.  rel-err ~1.3e-2.
"""

import numpy as np

import concourse.bass as bass
import concourse.bacc as bacc
import concourse.mybir as mybir
import concourse.tile as tile
from concourse.bass_utils import run_bass_kernel_spmd

F8 = mybir.dt.float8e4
F16 = mybir.dt.float16
F32 = mybir.dt.float32
AX = mybir.AxisListType
ALU = mybir.AluOpType
ACTF = mybir.ActivationFunctionType

B, S, E, H, U = 2048, 64, 512, 512, 512
N_CORES = 8
BL = B // N_CORES          # 256 rows per core
NCHUNK = BL // 128         # 2 chunks of 128 rows
S_GRP = 8                  # s rows per cast/transpose group
N_GRP = S // S_GRP
EC = E // 128              # 4 contraction chunks
E8 = E // 2                # fp8 e-range start (e >= E8 is fp8)
HC = H // 128

_LAST_RESULTS = {}


def build_kernel(reps: int = 1) -> bacc.Bacc:
    import os
    feat_slots = int(os.environ.get("FEAT_SLOTS", "12"))
    featt_bufs = int(os.environ.get("FEATT_BUFS", "4"))
    t16_bufs = int(os.environ.get("T16_BUFS", "12"))
    zt_bufs = int(os.environ.get("ZT_BUFS", "3"))
    n_ctx_pe = int(os.environ.get("N_CTX_PE", "16"))       # per-chunk PE ctx
    n_ctx_pe_tail = int(os.environ.get("N_CTX_PE_TAIL", "48"))  # last chunk
    ctx_blk = int(os.environ.get("CTX_BLK", "8"))          # DVE ctx tree block

    nc = bacc.Bacc(target_bir_lowering=False)

    feat_d = nc.declare_dram_parameter("features", [BL, S, E], F32, isOutput=False)
    hid_d = nc.declare_dram_parameter("hidden", [BL, H], F32, isOutput=False)
    w1i_d = nc.declare_dram_parameter("W1il", [128, 2, 2, U], F16, isOutput=False)
    b1_d = nc.declare_dram_parameter("b1", [U], F32, isOutput=False)
    w2_d = nc.declare_dram_parameter("W2", [H, U], F32, isOutput=False)
    b2_d = nc.declare_dram_parameter("b2", [U], F32, isOutput=False)
    wv_d = nc.declare_dram_parameter("Wv", [U, 1], F32, isOutput=False)
    id_d = nc.declare_dram_parameter("ident", [128, 128], F16, isOutput=False)
    out_d = nc.declare_dram_parameter("out", [BL, E], F32, isOutput=True)

    with tile.TileContext(nc) as tc:
        with (
            tc.tile_pool(name="const", bufs=1) as cpool,
            tc.tile_pool(name="featn", bufs=feat_slots) as fpool,
            tc.tile_pool(name="feat8", bufs=3) as f8pool,
            tc.tile_pool(name="featT", bufs=featt_bufs) as tpool,
            tc.tile_pool(name="work", bufs=2) as wpool,
            tc.tile_pool(name="tanh", bufs=t16_bufs) as hpool,
            tc.tile_pool(name="ctxp", bufs=2) as xpool,
            tc.tile_pool(name="pz", bufs=zt_bufs, space="PSUM") as pz,
            tc.tile_pool(name="ph2", bufs=1, space="PSUM") as ph2,
            tc.tile_pool(name="pctx", bufs=1, space="PSUM") as pctx,
        ):
            chunk_ids = [c for _ in range(reps) for c in range(NCHUNK)]
            n_steps = len(chunk_ids)

            feat16 = {}   # (step, group) -> fp16 tile [128, S_GRP, E]

            def issue_load(i, g):
                if (i, g) in feat16 or i >= n_steps:
                    return
                b0 = chunk_ids[i] * 128
                fg = fpool.tile([128, S_GRP, E], F16,
                                name=f"feat16_{i}_{g}", tag="feat16")
                nc.gpsimd.dma_start(
                    fg[:], feat_d[b0:b0 + 128, g * S_GRP:(g + 1) * S_GRP, :])
                feat16[(i, g)] = fg

            # first feature groups go ahead of the (gpsimd-queued) weight
            # loads so the PE pipeline fills as early as possible
            issue_load(0, 0)
            issue_load(0, 1)

            # ---- constants / weights ----
            # W1 arrives pre-interleaved from the host: W1il[j, r, i, u] =
            # W1[256r + 2j + i, u] (fp16) matching the fp8 pair transpose.
            w1i_sb = cpool.tile([128, 2, 2, U], F16)
            nc.sync.dma_start(w1i_sb[:], w1i_d[:])
            w2_sb = cpool.tile([128, HC, U], F16)
            nc.gpsimd.dma_start(w2_sb[:], w2_d.rearrange("(c p) u -> p c u", p=128))
            ident = cpool.tile([128, 128], F16)
            nc.sync.dma_start(ident[:], id_d[:])
            ones1 = cpool.tile([1, 128], F16)
            nc.vector.memset(ones1[:], 1.0)
            b1row = cpool.tile([1, U], F16)
            nc.gpsimd.dma_start(b1row[:], b1_d.rearrange("(one u) -> one u", one=1))
            b2row = cpool.tile([1, U], F16)
            nc.gpsimd.dma_start(b2row[:], b2_d.rearrange("(one u) -> one u", one=1))
            # wv twice in one row so one [128,1024] broadcast serves s-pairs
            wv2_row = cpool.tile([1, 2, U], F16)
            nc.gpsimd.dma_start(wv2_row[:, 0, :], wv_d.rearrange("u one -> one u"))
            nc.gpsimd.dma_start(wv2_row[:, 1, :], wv_d.rearrange("u one -> one u"))

            ps_wv = pz.tile([128, 1024], F32, tag="pz")
            nc.tensor.matmul(ps_wv[:, 0:512], ones1[:], wv2_row[:, 0, :],
                             start=True, stop=True)
            nc.tensor.matmul(ps_wv[:, 512:1024], ones1[:], wv2_row[:, 1, :],
                             start=True, stop=True)
            wv2_rep = cpool.tile([128, 1024], F16)
            nc.scalar.activation(wv2_rep[:], ps_wv[:], ACTF.Copy)

            for g in range(N_GRP):
                issue_load(0, g)
            issue_load(1, 0)
            issue_load(1, 1)

            feat8 = {}    # (step, group) -> fp8 tile [128, S_GRP, E]

            def issue_cast(i, g):
                # fp16 -> fp8 cast of the full e-range, on ACT
                if (i, g) in feat8 or i >= n_steps:
                    return
                f8 = f8pool.tile([128, S_GRP, E], F8,
                                 name=f"feat8_{i}_{g}", tag="feat8")
                nc.scalar.activation(f8[:], feat16[(i, g)][:], ACTF.Copy)
                feat8[(i, g)] = f8

            featT8 = {}   # (step, group) -> fp8-pair view [128, 2*S_GRP, 128, 2]

            def issue_transposes(i, g):
                f8t = tpool.tile([128, 2 * S_GRP, 128], F16,
                                 name=f"featT8_{i}_{g}", tag="featT8",
                                 bufs=featt_bufs)
                nc.sync.dma_start(f8t[:], feat8[(i, g)][:].bitcast(F16),
                                  transpose=True)
                featT8[(i, g)] = f8t[:].bitcast(F8).rearrange(
                    "p t (b two) -> p t b two", two=2)

            for i, c in enumerate(chunk_ids):
                b0 = c * 128
                last = (i == n_steps - 1)
                # casts for the first two groups (their loads landed first)
                issue_cast(i, 0)
                issue_cast(i, 1)

                # ---- h2 = hidden @ W2 + b1 + b2 (fp32 psum) ----
                hid16 = wpool.tile([128, H], F16, name=f"hid16_{i}", tag="hid16")
                nc.gpsimd.dma_start(hid16[:], hid_d[b0:b0 + 128, :])
                hidT = wpool.tile([128, HC, 128], F16, name=f"hidT_{i}", tag="hidT")
                nc.sync.dma_start(hidT[:], hid16[:], transpose=True)
                ps_h2 = ph2.tile([128, U], F32, tag="ph2")
                for k in range(HC):
                    nc.tensor.matmul(
                        ps_h2[:], hidT[:, k, :], w2_sb[:, k, :],
                        start=(k == 0), stop=False,
                    )
                nc.tensor.matmul(ps_h2[:], ones1[:], b1row[:], start=False, stop=False)
                nc.tensor.matmul(ps_h2[:], ones1[:], b2row[:], start=False, stop=True)
                h2_16 = wpool.tile([128, U], F16, name=f"h2_16_{i}", tag="h2_16")
                nc.scalar.activation(h2_16[:], ps_h2[:], ACTF.Copy)

                issue_transposes(i, 0)
                issue_transposes(i, 1)

                scores = wpool.tile([128, S], F32, name=f"scores_{i}", tag="scores")

                # ---- PE stream + tanh + scores, per s-pair ----
                for sp in range(S // 2):
                    s0 = sp * 2
                    g = s0 // S_GRP
                    if s0 % S_GRP == 0 and g + 2 < N_GRP:
                        # group boundary: stage group g+2 (cast + transposes)
                        issue_cast(i, g + 2)
                        issue_transposes(i, g + 2)
                    ps = pz.tile([128, 1024], F32, tag="pz")
                    for half in range(2):
                        s = s0 + half
                        sl = s % S_GRP
                        col = slice(half * 512, half * 512 + 512)
                        f8v = featT8[(i, g)]
                        first = True
                        for r in range(2):
                            for par in range(2):
                                nc.tensor.matmul(
                                    ps[:, col],
                                    f8v[:, sl * 2 + r, :, par],
                                    w1i_sb[:, r, par, :],
                                    start=first, stop=False,
                                )
                                first = False
                        nc.tensor.matmul(
                            ps[:, col], ident[:], h2_16[:],
                            start=False, stop=True,
                        )
                    t16 = hpool.tile([128, 1024], F16)
                    nc.scalar.activation(t16[:], ps[:], ACTF.Tanh)
                    for half in range(2):
                        s = s0 + half
                        dump = hpool.tile([128, 512], F16, tag="dump", bufs=2)
                        # scores[:, s] = sum_u t16 * wv  (STT accum; DVE 1x)
                        nc.vector.scalar_tensor_tensor(
                            out=dump[:],
                            in0=t16[:, half * 512: half * 512 + 512],
                            scalar=1.0,
                            in1=wv2_rep[:, half * 512: half * 512 + 512],
                            op0=ALU.mult, op1=ALU.mult,
                            accum_out=scores[:, s:s + 1],
                        )

                # ---- prefetch loads for the next chunk ----
                for g in range(2, N_GRP):
                    issue_load(i + 1, g)

                # ---- softmax over s (unnormalized; 1/Z applied at end) ----
                negmax = wpool.tile([128, 1], F32)
                nc.vector.tensor_reduce(
                    out=negmax[:], in_=scores[:], axis=AX.X, op=ALU.max, negate=True,
                )
                probs = wpool.tile([128, S], F32)
                zsum = wpool.tile([128, 1], F32)
                nc.scalar.activation(
                    probs[:], scores[:], ACTF.Exp,
                    bias=negmax[:], scale=1.0, accum_out=zsum[:],
                )
                rz = wpool.tile([128, 1], F32)
                nc.vector.reciprocal(rz[:], zsum[:])

                # ---- context: sum_s p_s * feat_s ----
                n_pe = n_ctx_pe_tail if last else n_ctx_pe
                n_dve = S - n_pe

                dve_accs = []
                for blk_start in range(0, n_dve, ctx_blk):
                    blk = list(range(blk_start, min(blk_start + ctx_blk, n_dve)))
                    acc = xpool.tile([128, E], F16, tag="ctxacc", bufs=10,
                                     name=f"ctxacc_{i}_{blk_start}")
                    for j, s in enumerate(blk):
                        g, sl = s // S_GRP, s % S_GRP
                        fslice = feat16[(i, g)][:, sl, :]
                        if j == 0:
                            nc.vector.tensor_scalar_mul(
                                acc[:], fslice, probs[:, s:s + 1])
                        else:
                            tmp = xpool.tile([128, E], F16, tag="ctxtmp", bufs=2,
                                             name=f"ctxtmp_{i}_{blk_start}_{j}")
                            nc.vector.tensor_scalar_mul(
                                tmp[:], fslice, probs[:, s:s + 1])
                            nc.vector.tensor_add(acc[:], acc[:], tmp[:])
                    dve_accs.append(acc)

                # PE part (last chunk only): diag(p_s) matmuls into PSUM
                if n_pe > 0:
                    ctx_ps = pctx.tile([128, E], F32, tag="pctx")
                    diags = []
                    for s in range(n_dve, S):
                        dg = xpool.tile([128, 128], F16, tag="diag", bufs=6,
                                        name=f"diag_{i}_{s}")
                        nc.vector.tensor_scalar_mul(
                            dg[:], ident[:], probs[:, s:s + 1])
                        diags.append(dg)
                    for j, s in enumerate(range(n_dve, S)):
                        g, sl = s // S_GRP, s % S_GRP
                        nc.tensor.matmul(
                            ctx_ps[:], diags[j][:], feat16[(i, g)][:, sl, :],
                            start=(j == 0), stop=(s == S - 1),
                        )
                    ctx_pe16 = xpool.tile([128, E], F16, tag="ctxpe",
                                          name=f"ctxpe_{i}")
                    nc.scalar.activation(ctx_pe16[:], ctx_ps[:], ACTF.Copy)
                    dve_accs.append(ctx_pe16)

                total = dve_accs[0]
                for acc in dve_accs[1:]:
                    nc.vector.tensor_add(total[:], total[:], acc[:])
                ctx16 = wpool.tile([128, E], F16, name=f"ctx16_{i}", tag="ctx16")
                nc.vector.tensor_scalar_mul(ctx16[:], total[:], rz[:])
                nc.gpsimd.dma_start(out_d[b0:b0 + 128, :], ctx16[:])

                # chunk-after-next's first two groups (slots recycle groups
                # this chunk has fully consumed)
                issue_load(i + 2, 0)
                issue_load(i + 2, 1)

    nc.compile()
    return nc


def _prep_w1il(W1: np.ndarray) -> np.ndarray:
    # W1il[j, r, i, u] = W1[256r + 2j + i, u] as fp16 (pair-interleave to
    # match the fp16-atom transpose of fp8 pairs)
    il = W1.reshape(2, 128, 2, U).transpose(1, 0, 2, 3)   # [j, r, i, u]
    return np.ascontiguousarray(il.astype(np.float16))


def kernel(**inputs) -> np.ndarray:
    features = np.ascontiguousarray(np.asarray(inputs["features"], dtype=np.float32))
    hidden = np.ascontiguousarray(np.asarray(inputs["hidden"], dtype=np.float32))
    W1 = np.ascontiguousarray(np.asarray(inputs["W1"], dtype=np.float32))
    b1 = np.ascontiguousarray(np.asarray(inputs["b1"], dtype=np.float32))
    W2 = np.ascontiguousarray(np.asarray(inputs["W2"], dtype=np.float32))
    b2 = np.ascontiguousarray(np.asarray(inputs["b2"], dtype=np.float32))
    Wv = np.ascontiguousarray(np.asarray(inputs["Wv"], dtype=np.float32))
    # bv shifts every score equally; softmax is invariant to it.

    nc = build_kernel()
    ident = np.eye(128, dtype=np.float16)
    w1il = _prep_w1il(W1)
    in_maps = []
    for i in range(N_CORES):
        in_maps.append({
            "features": features[i * BL:(i + 1) * BL],
            "hidden": hidden[i * BL:(i + 1) * BL],
            "W1il": w1il, "b1": b1, "W2": W2, "b2": b2, "Wv": Wv,
            "ident": ident,
        })
    import os as _os
    _tmpdir = _os.environ.get("BASS_TMPDIR") or None
    res = run_bass_kernel_spmd(nc, in_maps, core_ids=list(range(N_CORES)),
                               tmpdir=_tmpdir)
    _LAST_RESULTS["res"] = res
    if res.exec_time_ns is not None:
        print(f"HW exec time: {res.exec_time_ns} ns")
    out = np.concatenate([res.results[i]["out"] for i in range(N_CORES)], axis=0)
    return out.astype(np.float32)


# revision 16
# speedup vs baseline: 1.1020x; 1.0277x over previous
"""Bahdanau additive attention on 8 TRN2 NeuronCores, pure data parallel.

Per core (B_loc = 256 batch rows, two 128-row chunks):
  h1 = features @ W1                    -- fp16 (e<256) + fp8-feat x fp16-W1
                                           (e>=256) matmuls, fp32 PSUM accum
  z  = h1 + hidden@W2 + b1 + b2         -- h2/biases folded into PSUM via PE
  t  = tanh(z)                          -- ACT, per s-pair [128,1024]
  scores = t @ Wv                       -- DVE TT-mult (2x) + TS accum (4x)
  p  = exp(scores - max)                -- unnormalized softmax weights
  ctx = (sum_s p_s * feat_s) / Z        -- DVE TS(4x) products + TT(2x) tree;
                                           last chunk partly via PE diag-MMs
                                           to hide the pipeline tail.

DMA is the binding resource: features are cast-loaded fp32->fp16 once
(HBM floor ~94us/core), and the on-chip xbar transposes (256B-packet
limited) are trimmed by transposing the top e-half as fp8 pairs packed
in fp16 atoms.  The fp8 e-range pairs (e=256+2j, 256+2j+1) land on
partition j; the matching W1 rows are pair-interleaved on the host
(W1bot input, fp16) so contraction stays consistent.  rel-err ~1.3e-2.
"""

import numpy as np

import concourse.bass as bass
import concourse.bacc as bacc
import concourse.mybir as mybir
import concourse.tile as tile
from concourse.bass_utils import run_bass_kernel_spmd

F8 = mybir.dt.float8e4
F16 = mybir.dt.float16
F32 = mybir.dt.float32
AX = mybir.AxisListType
ALU = mybir.AluOpType
ACTF = mybir.ActivationFunctionType

B, S, E, H, U = 2048, 64, 512, 512, 512
N_CORES = 8
BL = B // N_CORES          # 256 rows per core
NCHUNK = BL // 128         # 2 chunks of 128 rows
S_GRP = 8                  # s rows per cast/transpose group
N_GRP = S // S_GRP
EC = E // 128              # 4 contraction chunks
E8 = E // 2                # fp8 e-range start (e >= E8 is fp8)
HC = H // 128

_LAST_RESULTS = {}


def build_kernel(reps: int = 1) -> bacc.Bacc:
    import os
    feat_slots = int(os.environ.get("FEAT_SLOTS", "12"))
    featt_bufs = int(os.environ.get("FEATT_BUFS", "4"))
    t16_bufs = int(os.environ.get("T16_BUFS", "16"))
    zt_bufs = int(os.environ.get("ZT_BUFS", "3"))
    n_ctx_pe = int(os.environ.get("N_CTX_PE", "16"))       # per-chunk PE ctx
    n_ctx_pe_tail = int(os.environ.get("N_CTX_PE_TAIL", "48"))  # last chunk
    ctx_blk = int(os.environ.get("CTX_BLK", "8"))          # DVE ctx tree block

    nc = bacc.Bacc(target_bir_lowering=False)

    feat_d = nc.declare_dram_parameter("features", [BL, S, E], F32, isOutput=False)
    hid_d = nc.declare_dram_parameter("hidden", [BL, H], F32, isOutput=False)
    w1i_d = nc.declare_dram_parameter("W1il", [128, 2, 2, U], F16, isOutput=False)
    b1_d = nc.declare_dram_parameter("b1", [U], F32, isOutput=False)
    w2_d = nc.declare_dram_parameter("W2", [H, U], F32, isOutput=False)
    b2_d = nc.declare_dram_parameter("b2", [U], F32, isOutput=False)
    wv_d = nc.declare_dram_parameter("Wv", [U, 1], F32, isOutput=False)
    id_d = nc.declare_dram_parameter("ident", [128, 128], F16, isOutput=False)
    out_d = nc.declare_dram_parameter("out", [BL, E], F32, isOutput=True)

    with tile.TileContext(nc) as tc:
        with (
            tc.tile_pool(name="const", bufs=1) as cpool,
            tc.tile_pool(name="featn", bufs=feat_slots) as fpool,
            tc.tile_pool(name="feat8", bufs=3) as f8pool,
            tc.tile_pool(name="featT", bufs=featt_bufs) as tpool,
            tc.tile_pool(name="work", bufs=2) as wpool,
            tc.tile_pool(name="tanh", bufs=t16_bufs) as hpool,
            tc.tile_pool(name="ctxp", bufs=2) as xpool,
            tc.tile_pool(name="pz", bufs=zt_bufs, space="PSUM") as pz,
            tc.tile_pool(name="ph2", bufs=1, space="PSUM") as ph2,
            tc.tile_pool(name="pctx", bufs=1, space="PSUM") as pctx,
        ):
            chunk_ids = [c for _ in range(reps) for c in range(NCHUNK)]
            n_steps = len(chunk_ids)

            feat16 = {}   # (step, group) -> fp16 tile [128, S_GRP, E]

            def issue_load(i, g):
                if (i, g) in feat16 or i >= n_steps:
                    return
                b0 = chunk_ids[i] * 128
                fg = fpool.tile([128, S_GRP, E], F16,
                                name=f"feat16_{i}_{g}", tag="feat16")
                nc.gpsimd.dma_start(
                    fg[:], feat_d[b0:b0 + 128, g * S_GRP:(g + 1) * S_GRP, :])
                feat16[(i, g)] = fg

            hid16s = {}

            def issue_hid(i):
                if i in hid16s or i >= n_steps:
                    return
                b0 = chunk_ids[i] * 128
                ht = wpool.tile([128, H], F16, name=f"hid16_{i}", tag="hid16")
                nc.gpsimd.dma_start(ht[:], hid_d[b0:b0 + 128, :])
                hid16s[i] = ht

            # hidden (small) and the first feature groups go ahead of the
            # (gpsimd-queued) weight loads so the PE pipeline fills early
            issue_hid(0)
            issue_load(0, 0)
            issue_load(0, 1)

            # ---- constants / weights ----
            # W1 arrives pre-interleaved from the host: W1il[j, r, i, u] =
            # W1[256r + 2j + i, u] (fp16) matching the fp8 pair transpose.
            w1i_sb = cpool.tile([128, 2, 2, U], F16)
            nc.sync.dma_start(w1i_sb[:], w1i_d[:])
            w2_sb = cpool.tile([128, HC, U], F16)
            nc.gpsimd.dma_start(w2_sb[:], w2_d.rearrange("(c p) u -> p c u", p=128))
            ident = cpool.tile([128, 128], F16)
            nc.sync.dma_start(ident[:], id_d[:])
            ones1 = cpool.tile([1, 128], F16)
            nc.vector.memset(ones1[:], 1.0)
            b1row = cpool.tile([1, U], F16)
            nc.gpsimd.dma_start(b1row[:], b1_d.rearrange("(one u) -> one u", one=1))
            b2row = cpool.tile([1, U], F16)
            nc.gpsimd.dma_start(b2row[:], b2_d.rearrange("(one u) -> one u", one=1))
            # wv twice in one row so one [128,1024] broadcast serves s-pairs
            wv2_row = cpool.tile([1, 2, U], F16)
            nc.gpsimd.dma_start(wv2_row[:, 0, :], wv_d.rearrange("u one -> one u"))
            nc.gpsimd.dma_start(wv2_row[:, 1, :], wv_d.rearrange("u one -> one u"))

            ps_wv = pz.tile([128, 1024], F32, tag="pz")
            nc.tensor.matmul(ps_wv[:, 0:512], ones1[:], wv2_row[:, 0, :],
                             start=True, stop=True)
            nc.tensor.matmul(ps_wv[:, 512:1024], ones1[:], wv2_row[:, 1, :],
                             start=True, stop=True)
            wv2_rep = cpool.tile([128, 1024], F16)
            nc.scalar.activation(wv2_rep[:], ps_wv[:], ACTF.Copy)

            for g in range(N_GRP):
                issue_load(0, g)
            issue_load(1, 0)
            issue_load(1, 1)

            feat8 = {}    # (step, group) -> fp8 tile [128, S_GRP, E]

            def issue_cast(i, g):
                # fp16 -> fp8 cast of the full e-range, on ACT
                if (i, g) in feat8 or i >= n_steps:
                    return
                f8 = f8pool.tile([128, S_GRP, E], F8,
                                 name=f"feat8_{i}_{g}", tag="feat8")
                nc.scalar.activation(f8[:], feat16[(i, g)][:], ACTF.Copy)
                feat8[(i, g)] = f8

            featT8 = {}   # (step, group) -> fp8-pair view [128, 2*S_GRP, 128, 2]

            def issue_transposes(i, g):
                f8t = tpool.tile([128, 2 * S_GRP, 128], F16,
                                 name=f"featT8_{i}_{g}", tag="featT8",
                                 bufs=featt_bufs)
                nc.sync.dma_start(f8t[:], feat8[(i, g)][:].bitcast(F16),
                                  transpose=True)
                featT8[(i, g)] = f8t[:].bitcast(F8).rearrange(
                    "p t (b two) -> p t b two", two=2)

            for i, c in enumerate(chunk_ids):
                b0 = c * 128
                last = (i == n_steps - 1)
                # casts for the first two processed groups
                first_two = [2, 3] if i == 0 else [0, 1]
                issue_cast(i, first_two[0])
                issue_cast(i, first_two[1])

                # ---- h2 = hidden @ W2 + b1 + b2 (fp32 psum) ----
                issue_hid(i)
                hidT = wpool.tile([128, HC, 128], F16, name=f"hidT_{i}", tag="hidT")
                nc.sync.dma_start(hidT[:], hid16s[i][:], transpose=True)
                ps_h2 = ph2.tile([128, U], F32, tag="ph2")
                for k in range(HC):
                    nc.tensor.matmul(
                        ps_h2[:], hidT[:, k, :], w2_sb[:, k, :],
                        start=(k == 0), stop=False,
                    )
                nc.tensor.matmul(ps_h2[:], ones1[:], b1row[:], start=False, stop=False)
                nc.tensor.matmul(ps_h2[:], ones1[:], b2row[:], start=False, stop=True)
                h2_16 = wpool.tile([128, U], F16, name=f"h2_16_{i}", tag="h2_16")
                nc.scalar.activation(h2_16[:], ps_h2[:], ACTF.Copy)

                gorder = [(g + 2) % N_GRP for g in range(N_GRP)] if i == 0 \
                    else list(range(N_GRP))
                issue_transposes(i, gorder[0])
                issue_transposes(i, gorder[1])

                scores = wpool.tile([128, S], F32, name=f"scores_{i}", tag="scores")

                # ---- PE stream + tanh + scores, per s-pair ----
                for sp in range(S // 2):
                    gi, poff = divmod(sp, S_GRP // 2)
                    g = gorder[gi]
                    s0 = g * S_GRP + poff * 2
                    if poff == 0 and gi + 2 < N_GRP:
                        # group boundary: stage 2 groups ahead
                        issue_cast(i, gorder[gi + 2])
                        issue_transposes(i, gorder[gi + 2])
                    ps = pz.tile([128, 1024], F32, tag="pz")
                    for half in range(2):
                        s = s0 + half
                        sl = s % S_GRP
                        col = slice(half * 512, half * 512 + 512)
                        f8v = featT8[(i, g)]
                        first = True
                        for r in range(2):
                            for par in range(2):
                                nc.tensor.matmul(
                                    ps[:, col],
                                    f8v[:, sl * 2 + r, :, par],
                                    w1i_sb[:, r, par, :],
                                    start=first, stop=False,
                                )
                                first = False
                        nc.tensor.matmul(
                            ps[:, col], ident[:], h2_16[:],
                            start=False, stop=True,
                        )
                    t16 = hpool.tile([128, 1024], F16)
                    nc.scalar.activation(t16[:], ps[:], ACTF.Tanh)
                    for half in range(2):
                        s = s0 + half
                        dump = hpool.tile([128, 512], F16, tag="dump", bufs=2)
                        # scores[:, s] = sum_u t16 * wv  (STT accum; DVE 1x)
                        nc.vector.scalar_tensor_tensor(
                            out=dump[:],
                            in0=t16[:, half * 512: half * 512 + 512],
                            scalar=1.0,
                            in1=wv2_rep[:, half * 512: half * 512 + 512],
                            op0=ALU.mult, op1=ALU.mult,
                            accum_out=scores[:, s:s + 1],
                        )

                # ---- prefetch loads for the next chunk ----
                issue_hid(i + 1)
                for g in range(2, N_GRP):
                    issue_load(i + 1, g)

                # ---- softmax over s (unnormalized; 1/Z applied at end) ----
                negmax = wpool.tile([128, 1], F32)
                nc.vector.tensor_reduce(
                    out=negmax[:], in_=scores[:], axis=AX.X, op=ALU.max, negate=True,
                )
                probs = wpool.tile([128, S], F32)
                zsum = wpool.tile([128, 1], F32)
                nc.scalar.activation(
                    probs[:], scores[:], ACTF.Exp,
                    bias=negmax[:], scale=1.0, accum_out=zsum[:],
                )
                rz = wpool.tile([128, 1], F32)
                nc.vector.reciprocal(rz[:], zsum[:])

                # ---- context: sum_s p_s * feat_s ----
                n_pe = n_ctx_pe_tail if last else n_ctx_pe
                n_dve = S - n_pe

                dve_accs = []
                for blk_start in range(0, n_dve, ctx_blk):
                    blk = list(range(blk_start, min(blk_start + ctx_blk, n_dve)))
                    acc = xpool.tile([128, E], F16, tag="ctxacc", bufs=10,
                                     name=f"ctxacc_{i}_{blk_start}")
                    for j, s in enumerate(blk):
                        g, sl = s // S_GRP, s % S_GRP
                        fslice = feat16[(i, g)][:, sl, :]
                        if j == 0:
                            nc.vector.tensor_scalar_mul(
                                acc[:], fslice, probs[:, s:s + 1])
                        else:
                            tmp = xpool.tile([128, E], F16, tag="ctxtmp", bufs=2,
                                             name=f"ctxtmp_{i}_{blk_start}_{j}")
                            nc.vector.tensor_scalar_mul(
                                tmp[:], fslice, probs[:, s:s + 1])
                            nc.vector.tensor_add(acc[:], acc[:], tmp[:])
                    dve_accs.append(acc)

                # PE part (last chunk only): diag(p_s) matmuls into PSUM
                if n_pe > 0:
                    ctx_ps = pctx.tile([128, E], F32, tag="pctx")
                    diags = []
                    for s in range(n_dve, S):
                        dg = xpool.tile([128, 128], F16, tag="diag", bufs=6,
                                        name=f"diag_{i}_{s}")
                        nc.vector.tensor_scalar_mul(
                            dg[:], ident[:], probs[:, s:s + 1])
                        diags.append(dg)
                    for j, s in enumerate(range(n_dve, S)):
                        g, sl = s // S_GRP, s % S_GRP
                        nc.tensor.matmul(
                            ctx_ps[:], diags[j][:], feat16[(i, g)][:, sl, :],
                            start=(j == 0), stop=(s == S - 1),
                        )
                    ctx_pe16 = xpool.tile([128, E], F16, tag="ctxpe",
                                          name=f"ctxpe_{i}")
                    nc.scalar.activation(ctx_pe16[:], ctx_ps[:], ACTF.Copy)
                    dve_accs.append(ctx_pe16)

                total = dve_accs[0]
                for acc in dve_accs[1:]:
                    nc.vector.tensor_add(total[:], total[:], acc[:])
                ctx16 = wpool.tile([128, E], F16, name=f"ctx16_{i}", tag="ctx16")
                nc.vector.tensor_scalar_mul(ctx16[:], total[:], rz[:])
                nc.gpsimd.dma_start(out_d[b0:b0 + 128, :], ctx16[:])

                # chunk-after-next's first two groups (slots recycle groups
                # this chunk has fully consumed)
                issue_load(i + 2, 0)
                issue_load(i + 2, 1)

    nc.compile()
    return nc


def _prep_w1il(W1: np.ndarray) -> np.ndarray:
    # W1il[j, r, i, u] = W1[256r + 2j + i, u] as fp16 (pair-interleave to
    # match the fp16-atom transpose of fp8 pairs)
    il = W1.reshape(2, 128, 2, U).transpose(1, 0, 2, 3)   # [j, r, i, u]
    return np.ascontiguousarray(il.astype(np.float16))


def kernel(**inputs) -> np.ndarray:
    features = np.ascontiguousarray(np.asarray(inputs["features"], dtype=np.float32))
    hidden = np.ascontiguousarray(np.asarray(inputs["hidden"], dtype=np.float32))
    W1 = np.ascontiguousarray(np.asarray(inputs["W1"], dtype=np.float32))
    b1 = np.ascontiguousarray(np.asarray(inputs["b1"], dtype=np.float32))
    W2 = np.ascontiguousarray(np.asarray(inputs["W2"], dtype=np.float32))
    b2 = np.ascontiguousarray(np.asarray(inputs["b2"], dtype=np.float32))
    Wv = np.ascontiguousarray(np.asarray(inputs["Wv"], dtype=np.float32))
    # bv shifts every score equally; softmax is invariant to it.

    nc = build_kernel()
    ident = np.eye(128, dtype=np.float16)
    w1il = _prep_w1il(W1)
    in_maps = []
    for i in range(N_CORES):
        in_maps.append({
            "features": features[i * BL:(i + 1) * BL],
            "hidden": hidden[i * BL:(i + 1) * BL],
            "W1il": w1il, "b1": b1, "W2": W2, "b2": b2, "Wv": Wv,
            "ident": ident,
        })
    import os as _os
    _tmpdir = _os.environ.get("BASS_TMPDIR") or None
    res = run_bass_kernel_spmd(nc, in_maps, core_ids=list(range(N_CORES)),
                               tmpdir=_tmpdir)
    _LAST_RESULTS["res"] = res
    if res.exec_time_ns is not None:
        print(f"HW exec time: {res.exec_time_ns} ns")
    out = np.concatenate([res.results[i]["out"] for i in range(N_CORES)], axis=0)
    return out.astype(np.float32)
